# revision 1
# baseline (speedup 1.0000x reference)
"""Trainium2 kernel for nn_BFM_torch_56384330662315 (gnn_message_passing).

Reference semantics (B=4, C=128, N=2048, K=16):
  feats = transpose(seg_features, (0,2,1))                 # [B,N,C]
  per sample: adj = boundary-cut symmetric kNN graph; two GCN layers
  out = refined + feats

Each GCN layer computes ``out = (adj/deg) @ f + feat`` but returns plain
``feat`` whenever any node has zero degree (``has_zero`` in the reference).
Any node classified as a boundary node (argmax(edge_preds)==1) gets its row
AND column zeroed in the symmetric adjacency, so it has zero degree.  Hence
for every sample that has at least one edge node and at least one non-edge
node, both GCN layers are exact identities and the whole module reduces,
bit-for-bit in fp32, to:

  out = 2 * transpose(seg_features, (0,2,1))

The device kernel evaluates that scaled transpose, data-parallel over the
8 NeuronCores (each core transposes a [128, 1024] half-sample via the PE
transpose path).  The per-sample condition is checked on host from
edge_preds (tiny); samples that don't satisfy it (probability ~2^-2047
for the randn inputs this problem is generated with) fall back to an
exact numpy port of the reference.
"""

import os

import numpy as np

# recover cleanly if a previous kernel left a NeuronCore exec unit wedged
os.environ.setdefault("NEURON_RT_RESET_CORES", "1")

B, C, N, K = 4, 128, 2048, 16
GEO_FILL = 1000.0
NCORES = 8
NSH = N * B // NCORES  # 1024 columns of seg_features per core
NBLK = NSH // 128      # 8 transpose blocks per core

_EXEC = None  # cached (jitted shard_map, metadata)


def _build_nc():
    """Per-core program: y[NSH, C] = 2 * x[C, NSH]^T via PE transpose.

    Pipeline tuned against the TRN2 instruction cost model (TimelineSim):
    - 4 column chunks of [128, 256]; loads split HWDGE (SP x3) + SWDGE
      (gpsimd x1) so DMA-engine transfers pack back-to-back.
    - PE transposes each 128-col block into a per-chunk [128, 256] PSUM
      tile (identity built on-chip by gpsimd memset+affine_select;
      transpose mode requires a permutation matrix, so the x2 lives in
      the PSUM->SBUF copies).
    - ONE paired scale-copy per chunk (halves DVE serialization),
      alternating DVE / ACT so consecutive chunks' copies overlap.
    - stores per chunk on ACT / gpsimd / SP / SP to spread HWDGE issue.

    (A faster dma_scatter_add prepare/trigger store variant scored 8770ns
    in the cost model and passed CoreSim bit-exactly, but produced wrong
    values on real hardware -- deferred-scatter ordering that the
    simulators honor is not enforced by the silicon.  Kept off.)
    """
    import concourse.bass as bass
    import concourse.tile as tile
    from concourse import bacc, mybir
    from concourse.masks import make_identity

    F32 = mybir.dt.float32
    nc = bacc.Bacc(
        "TRN2",
        target_bir_lowering=False,
        debug=False,
        num_devices=NCORES,
    )
    x = nc.dram_tensor("x", [C, NSH], F32, kind="ExternalInput").ap()
    y = nc.dram_tensor("y", [NSH, C], F32, kind="ExternalOutput").ap()
    yv = y.rearrange("(i p) c -> p i c", p=128)  # [128, 8, 128]

    chunks = [(256, "sync"), (256, "gpsimd"), (256, "sync"), (256, "sync")]
    stores = ["scalar", "gpsimd", "sync", "sync"]
    copy_cycle = ["vector", "scalar", "vector", "scalar"]
    with tile.TileContext(nc) as tc:
        with (
            tc.tile_pool(name="const", bufs=1) as cpool,
            tc.tile_pool(name="xin", bufs=4) as ipool,
            tc.tile_pool(name="ps", bufs=4, space="PSUM") as ppool,
            tc.tile_pool(name="yout", bufs=4) as opool,
        ):
            idt = cpool.tile([128, 128], F32)

            # all loads first: the SWDGE load's descriptor gen on Pool then
            # precedes the identity build, so its transfer packs right after
            # the first HWDGE load (identity is still ready ~3.1us, just
            # before the first matmul's data arrives ~3.2us)
            xts = []
            col0 = 0
            for j, (cols, leng) in enumerate(chunks):
                xt = ipool.tile([128, cols], F32, tag=f"xt{j}")
                getattr(nc, leng).dma_start(xt[:], x[:, col0 : col0 + cols])
                xts.append(xt)
                col0 += cols
            make_identity(nc, idt[:])

            col0 = 0
            for j, (cols, leng) in enumerate(chunks):
                bper = cols // 128
                blk0 = col0 // 128
                xt = xts[j]
                ot = opool.tile([128, bper, 128], F32, tag="ot")
                ps = ppool.tile([128, bper * 128], F32, tag="ps")
                for ii in range(bper):
                    nc.tensor.matmul(
                        ps[:, bass.ts(ii, 128)],
                        xt[:, bass.ts(ii, 128)],
                        idt[:],
                        is_transpose=True,
                    )
                if copy_cycle[j % len(copy_cycle)] == "scalar":
                    nc.scalar.mul(ot[:], ps[:], 2.0)
                else:
                    nc.vector.tensor_scalar_mul(ot[:], ps[:], 2.0)
                getattr(nc, stores[j]).dma_start(
                    yv[:, blk0 : blk0 + bper, :], ot[:]
                )
                col0 += cols
    nc.compile()
    return nc


def _get_exec():
    """Build the per-core Bass program once and wrap it in a cached
    jit(shard_map) over the 8 cores (mirrors bass2jax.run_bass_via_pjrt)."""
    global _EXEC
    if _EXEC is not None:
        return _EXEC

    import jax
    from jax.experimental.shard_map import shard_map
    from jax.sharding import Mesh, PartitionSpec

    from concourse import bass2jax, mybir

    bass2jax.install_neuronx_cc_hook()
    nc = _build_nc()
    partition_name = nc.partition_id_tensor.name if nc.partition_id_tensor else None

    in_names: list = []
    out_names: list = []
    out_avals: list = []
    zero_outs: list = []
    for alloc in nc.m.functions[0].allocations:
        if not isinstance(alloc, mybir.MemoryLocationSet):
            continue
        name = alloc.memorylocations[0].name
        if alloc.kind == "ExternalInput":
            if name != partition_name:
                in_names.append(name)
        elif alloc.kind == "ExternalOutput":
            out_names.append(name)
            shape = tuple(alloc.tensor_shape)
            dtype = mybir.dt.np(alloc.dtype)
            out_avals.append(jax.core.ShapedArray(shape, dtype))
            zero_outs.append(np.zeros(shape, dtype))
    n_params = len(in_names)
    n_outs = len(out_avals)
    all_names = in_names + out_names
    if partition_name is not None:
        all_names.append(partition_name)

    def _body(*args):
        operands = list(args)
        if partition_name is not None:
            operands.append(bass2jax.partition_id_tensor())
        outs = bass2jax._bass_exec_p.bind(
            *operands,
            out_avals=tuple(out_avals),
            in_names=tuple(all_names),
            out_names=tuple(out_names),
            lowering_input_output_aliases=(),
            sim_require_finite=True,
            sim_require_nnan=True,
            nc=nc,
        )
        return tuple(outs)

    devices = jax.devices()[:NCORES]
    assert len(devices) == NCORES, f"need {NCORES} cores, have {len(jax.devices())}"
    mesh = Mesh(np.asarray(devices), ("core",))
    in_specs = (PartitionSpec("core"),) * (n_params + n_outs)
    out_specs = (PartitionSpec("core"),) * n_outs
    donate = tuple(range(n_params, n_params + n_outs))
    sharded = jax.jit(
        shard_map(
            _body, mesh=mesh, in_specs=in_specs, out_specs=out_specs, check_rep=False
        ),
        donate_argnums=donate,
        keep_unused=True,
    )
    _EXEC = (sharded, in_names, out_names, out_avals, zero_outs)
    return _EXEC


def _make_concat_inputs(seg: np.ndarray):
    """Per-core input shards, concatenated on axis 0 for shard_map."""
    xs = []
    for k in range(NCORES):
        b, h = k // 2, k % 2
        xs.append(seg[b, :, h * NSH : (h + 1) * NSH])
    return {"x": np.concatenate(xs, axis=0)}


def _run_device(seg: np.ndarray) -> np.ndarray:
    """seg [B,C,N] f32 -> 2*transpose [B,N,C] on the 8 cores, with retry and
    a bit-exact host fallback (2*transpose is exact in fp32 either way) in
    case a previous session left the accelerator wedged."""
    last_err = None
    for attempt in range(2):
        try:
            return _run_device_once(seg)
        except Exception as e:  # transient NRT_EXEC_UNIT_UNRECOVERABLE etc.
            last_err = e
    import sys

    print(
        f"kernel: device path failed twice ({type(last_err).__name__}: "
        f"{last_err}); computing on host (bit-identical result)",
        file=sys.stderr,
    )
    return np.ascontiguousarray(2.0 * seg.transpose(0, 2, 1))


def _run_device_once(seg: np.ndarray) -> np.ndarray:
    sharded, in_names, out_names, out_avals, zero_outs = _get_exec()
    by_name = _make_concat_inputs(seg)
    concat_in = [by_name[n] for n in in_names]
    concat_zeros = [
        np.zeros((NCORES * z.shape[0], *z.shape[1:]), z.dtype) for z in zero_outs
    ]
    out_arrs = sharded(*concat_in, *concat_zeros)
    y = np.asarray(out_arrs[out_names.index("y")]).reshape(NCORES, NSH, C)

    out = np.empty((B, N, C), dtype=np.float32)
    for k in range(NCORES):
        b, h = k // 2, k % 2
        out[b, h * NSH : (h + 1) * NSH, :] = y[k]
    return out


# ---------------------------------------------------------------------------
# Exact numpy port of the reference — fallback for samples where the GCN does
# not collapse to identity (never hit for this problem's input distribution).
# ---------------------------------------------------------------------------


def _np_build_adj(g, edge_cls, k):
    n = g.shape[0]
    nbrs = np.argsort(g, axis=-1, kind="stable")[:, :k]
    rows = np.arange(n)[:, None]
    adj = np.zeros((n, n), g.dtype)
    adj[rows, nbrs] = 1.0
    adj[nbrs, rows] = 1.0
    is_edge = edge_cls == 1
    adj = np.where(is_edge[:, None], 0.0, adj)
    edge_col = is_edge[None, :]
    cond = (adj == 1) & edge_col
    maxgeo = np.min(np.where(cond, g, GEO_FILL), axis=-1)
    adjr = np.where(g > maxgeo[:, None], 0.0, adj)
    adjr = np.where(edge_col, 0.0, adjr)
    adj2 = np.where(is_edge[:, None], 0.0, adjr)
    adj_sym = ((adj2 > 0) | (adj2.T > 0)).astype(g.dtype)
    if np.all(is_edge):
        return np.eye(n, dtype=g.dtype)
    return adj_sym


def _np_gcn(feat, adj, W, b):
    identity = feat
    f = np.maximum(feat @ W.T + b, 0.0).astype(np.float32)
    row_deg = np.sum(adj, axis=-1, keepdims=True)
    col_deg = np.sum(adj, axis=-2, keepdims=True)
    degree = np.sqrt(row_deg) @ np.sqrt(col_deg)
    if np.any(degree == 0):
        return identity
    out = (adj / degree) @ f + identity
    return out.astype(np.float32)


def _np_sample(feat, ep, g, W1, b1, W2, b2):
    edge_cls = np.argmax(ep, axis=0)
    adj = _np_build_adj(g, edge_cls, K)
    r = _np_gcn(feat, adj, W1, b1)
    r = _np_gcn(r, adj, W2, b2)
    return r


def kernel(**inputs) -> np.ndarray:
    seg = np.ascontiguousarray(np.asarray(inputs["seg_features"], dtype=np.float32))
    ep = np.asarray(inputs["edge_preds"], dtype=np.float32)

    # argmax over the 2 class logits: class 1 iff ep[1] > ep[0] (ties -> 0)
    edge = ep[:, 1, :] > ep[:, 0, :]
    any_e = edge.any(axis=1)
    all_e = edge.all(axis=1)
    fast = any_e & ~all_e  # GCN layers are exact identities

    out = _run_device(seg)  # 2 * transpose, correct wherever fast[b]

    if not fast.all():
        g_all = np.asarray(inputs["gmatrix"], dtype=np.float32)
        W1 = np.asarray(inputs["W1"], dtype=np.float32)
        b1 = np.asarray(inputs["b1"], dtype=np.float32)
        W2 = np.asarray(inputs["W2"], dtype=np.float32)
        b2 = np.asarray(inputs["b2"], dtype=np.float32)
        for b in range(B):
            if not fast[b]:
                feat = np.ascontiguousarray(seg[b].T)
                r = _np_sample(feat, ep[b], g_all[b], W1, b1, W2, b2)
                out[b] = r + feat
    return out



# revision 2
# speedup vs baseline: 1.1693x; 1.1693x over previous
"""Trainium2 kernel for nn_BFM_torch_56384330662315 (gnn_message_passing).

Reference semantics (B=4, C=128, N=2048, K=16):
  feats = transpose(seg_features, (0,2,1))                 # [B,N,C]
  per sample: adj = boundary-cut symmetric kNN graph; two GCN layers
  out = refined + feats

Each GCN layer computes ``out = (adj/deg) @ f + feat`` but returns plain
``feat`` whenever any node has zero degree (``has_zero`` in the reference).
Any node classified as a boundary node (argmax(edge_preds)==1) gets its row
AND column zeroed in the symmetric adjacency, so it has zero degree.  Hence
for every sample that has at least one edge node and at least one non-edge
node, both GCN layers are exact identities and the whole module reduces,
bit-for-bit in fp32, to:

  out = 2 * transpose(seg_features, (0,2,1))

The device kernel evaluates that scaled transpose, data-parallel over the
8 NeuronCores.  Each core handles a [128, 1024] half-sample in bfloat16
(rel err ~1e-3, well under the 2e-2 gate):

  - one DMA-transpose (XBAR) load brings x^T into SBUF already transposed,
    split [6,2] blocks so the second transfer pipelines behind the first
    HWDGE descriptor generation;
  - DVE applies the x2 (the only arithmetic left);
  - two HWDGE stores write the result via 2KB/partition descriptors into a
    packed [128, 8*128] DRAM layout (the host unpermutes, a pure reshape).

The per-sample condition is checked on host from edge_preds (tiny);
samples that don't satisfy it (probability ~2^-2047 for the randn inputs
this problem is generated with) fall back to an exact numpy port of the
reference.
"""

import os

import numpy as np

# recover cleanly if a previous kernel left a NeuronCore exec unit wedged
os.environ.setdefault("NEURON_RT_RESET_CORES", "1")

B, C, N, K = 4, 128, 2048, 16
GEO_FILL = 1000.0
NCORES = 8
NSH = N * B // NCORES  # 1024 columns of seg_features per core
NBLK = NSH // 128      # 8 transpose blocks per core

# per-core program shape (sweep-tuned in TimelineSim)
XBAR_SPLIT = [6, 2]
SCALE_SPLIT = [6, 2]
STORE_SPLIT = [6, 2]

_EXEC = None  # cached (jitted shard_map, metadata)


def _bf16():
    import ml_dtypes

    return np.dtype(ml_dtypes.bfloat16)


def _build_nc():
    """Per-core program: y[128, 8*128] = packed 2*x[128,1024]^T in bf16.

    y[p, i*128 + c] = 2 * x[c, i*128 + p]  (host unpermutes blocks).

    Pipeline (TimelineSim-tuned): XBAR dma-transpose loads split [6,2] so
    the second transfer packs right behind the first (HWDGE descriptor
    generations serialize at ~625ns each); the x2 scale runs on DVE (3x
    faster than ACT for this shape); stores split [6,2] on the SP queue so
    the first store's descriptor generation overlaps the second scale.
    """
    import concourse.tile as tile
    from concourse import bacc, mybir

    BF16 = mybir.dt.bfloat16
    nc = bacc.Bacc(
        "TRN2",
        target_bir_lowering=False,
        debug=False,
        num_devices=NCORES,
    )
    x = nc.dram_tensor("x", [C, NSH], BF16, kind="ExternalInput").ap()
    y = nc.dram_tensor("y", [128, NSH], BF16, kind="ExternalOutput").ap()

    with tile.TileContext(nc) as tc:
        with (
            tc.tile_pool(name="xin", bufs=1) as ipool,
            tc.tile_pool(name="yout", bufs=1) as opool,
        ):
            xt = ipool.tile([128, NBLK, 128], BF16, tag="xt")
            b0 = 0
            for nb in XBAR_SPLIT:
                nc.sync.dma_start(
                    xt[:, b0 : b0 + nb, :],
                    x[:, b0 * 128 : (b0 + nb) * 128],
                    transpose=True,
                )
                b0 += nb
            ot = opool.tile([128, NBLK, 128], BF16, tag="ot")
            b0 = 0
            for nb in SCALE_SPLIT:
                nc.vector.tensor_scalar_mul(
                    ot[:, b0 : b0 + nb, :], xt[:, b0 : b0 + nb, :], 2.0
                )
                b0 += nb
            b0 = 0
            for nb in STORE_SPLIT:
                nc.sync.dma_start(
                    y[:, b0 * 128 : (b0 + nb) * 128],
                    ot[:, b0 : b0 + nb, :],
                )
                b0 += nb
    nc.compile()
    return nc


def _get_exec():
    """Build the per-core Bass program once and wrap it in a cached
    jit(shard_map) over the 8 cores (mirrors bass2jax.run_bass_via_pjrt)."""
    global _EXEC
    if _EXEC is not None:
        return _EXEC

    import jax
    from jax.experimental.shard_map import shard_map
    from jax.sharding import Mesh, PartitionSpec

    from concourse import bass2jax, mybir

    bass2jax.install_neuronx_cc_hook()
    nc = _build_nc()
    partition_name = nc.partition_id_tensor.name if nc.partition_id_tensor else None

    in_names: list = []
    out_names: list = []
    out_avals: list = []
    zero_outs: list = []
    for alloc in nc.m.functions[0].allocations:
        if not isinstance(alloc, mybir.MemoryLocationSet):
            continue
        name = alloc.memorylocations[0].name
        if alloc.kind == "ExternalInput":
            if name != partition_name:
                in_names.append(name)
        elif alloc.kind == "ExternalOutput":
            out_names.append(name)
            shape = tuple(alloc.tensor_shape)
            dtype = mybir.dt.np(alloc.dtype)
            out_avals.append(jax.core.ShapedArray(shape, dtype))
            zero_outs.append(np.zeros(shape, dtype))
    n_params = len(in_names)
    n_outs = len(out_avals)
    all_names = in_names + out_names
    if partition_name is not None:
        all_names.append(partition_name)

    def _body(*args):
        operands = list(args)
        if partition_name is not None:
            operands.append(bass2jax.partition_id_tensor())
        outs = bass2jax._bass_exec_p.bind(
            *operands,
            out_avals=tuple(out_avals),
            in_names=tuple(all_names),
            out_names=tuple(out_names),
            lowering_input_output_aliases=(),
            sim_require_finite=True,
            sim_require_nnan=True,
            nc=nc,
        )
        return tuple(outs)

    devices = jax.devices()[:NCORES]
    assert len(devices) == NCORES, f"need {NCORES} cores, have {len(jax.devices())}"
    mesh = Mesh(np.asarray(devices), ("core",))
    in_specs = (PartitionSpec("core"),) * (n_params + n_outs)
    out_specs = (PartitionSpec("core"),) * n_outs
    donate = tuple(range(n_params, n_params + n_outs))
    sharded = jax.jit(
        shard_map(
            _body, mesh=mesh, in_specs=in_specs, out_specs=out_specs, check_rep=False
        ),
        donate_argnums=donate,
        keep_unused=True,
    )
    _EXEC = (sharded, in_names, out_names, out_avals, zero_outs)
    return _EXEC


def _make_concat_inputs(seg: np.ndarray):
    """Per-core bf16 input shards, concatenated on axis 0 for shard_map."""
    bf16 = _bf16()
    xs = []
    for k in range(NCORES):
        b, h = k // 2, k % 2
        xs.append(seg[b, :, h * NSH : (h + 1) * NSH].astype(bf16))
    return {"x": np.concatenate(xs, axis=0)}


def _run_device(seg: np.ndarray) -> np.ndarray:
    """seg [B,C,N] f32 -> 2*transpose [B,N,C] f32 on the 8 cores (bf16
    internally, rel err ~1e-3), with retry and a host fallback in case a
    previous session left the accelerator wedged."""
    last_err = None
    for attempt in range(2):
        try:
            return _run_device_once(seg)
        except Exception as e:  # transient NRT_EXEC_UNIT_UNRECOVERABLE etc.
            last_err = e
    import sys

    print(
        f"kernel: device path failed twice ({type(last_err).__name__}: "
        f"{last_err}); computing on host",
        file=sys.stderr,
    )
    return np.ascontiguousarray(2.0 * seg.transpose(0, 2, 1))


def _run_device_once(seg: np.ndarray) -> np.ndarray:
    sharded, in_names, out_names, out_avals, zero_outs = _get_exec()
    by_name = _make_concat_inputs(seg)
    concat_in = [by_name[n] for n in in_names]
    concat_zeros = [
        np.zeros((NCORES * z.shape[0], *z.shape[1:]), z.dtype) for z in zero_outs
    ]
    out_arrs = sharded(*concat_in, *concat_zeros)
    y = np.asarray(out_arrs[out_names.index("y")]).reshape(NCORES, 128, NBLK, 128)

    out = np.empty((B, N, C), dtype=np.float32)
    for k in range(NCORES):
        b, h = k // 2, k % 2
        # y[k][p, i, c] = 2*x[c, i*128+p] -> rows i*128+p of the half-sample
        out[b, h * NSH : (h + 1) * NSH, :] = (
            y[k].transpose(1, 0, 2).reshape(NSH, C).astype(np.float32)
        )
    return out


# ---------------------------------------------------------------------------
# Exact numpy port of the reference — fallback for samples where the GCN does
# not collapse to identity (never hit for this problem's input distribution).
# ---------------------------------------------------------------------------


def _np_build_adj(g, edge_cls, k):
    n = g.shape[0]
    nbrs = np.argsort(g, axis=-1, kind="stable")[:, :k]
    rows = np.arange(n)[:, None]
    adj = np.zeros((n, n), g.dtype)
    adj[rows, nbrs] = 1.0
    adj[nbrs, rows] = 1.0
    is_edge = edge_cls == 1
    adj = np.where(is_edge[:, None], 0.0, adj)
    edge_col = is_edge[None, :]
    cond = (adj == 1) & edge_col
    maxgeo = np.min(np.where(cond, g, GEO_FILL), axis=-1)
    adjr = np.where(g > maxgeo[:, None], 0.0, adj)
    adjr = np.where(edge_col, 0.0, adjr)
    adj2 = np.where(is_edge[:, None], 0.0, adjr)
    adj_sym = ((adj2 > 0) | (adj2.T > 0)).astype(g.dtype)
    if np.all(is_edge):
        return np.eye(n, dtype=g.dtype)
    return adj_sym


def _np_gcn(feat, adj, W, b):
    identity = feat
    f = np.maximum(feat @ W.T + b, 0.0).astype(np.float32)
    row_deg = np.sum(adj, axis=-1, keepdims=True)
    col_deg = np.sum(adj, axis=-2, keepdims=True)
    degree = np.sqrt(row_deg) @ np.sqrt(col_deg)
    if np.any(degree == 0):
        return identity
    out = (adj / degree) @ f + identity
    return out.astype(np.float32)


def _np_sample(feat, ep, g, W1, b1, W2, b2):
    edge_cls = np.argmax(ep, axis=0)
    adj = _np_build_adj(g, edge_cls, K)
    r = _np_gcn(feat, adj, W1, b1)
    r = _np_gcn(r, adj, W2, b2)
    return r


def kernel(**inputs) -> np.ndarray:
    seg = np.ascontiguousarray(np.asarray(inputs["seg_features"], dtype=np.float32))
    ep = np.asarray(inputs["edge_preds"], dtype=np.float32)

    # argmax over the 2 class logits: class 1 iff ep[1] > ep[0] (ties -> 0)
    edge = ep[:, 1, :] > ep[:, 0, :]
    any_e = edge.any(axis=1)
    all_e = edge.all(axis=1)
    fast = any_e & ~all_e  # GCN layers are exact identities

    out = _run_device(seg)  # 2 * transpose, correct wherever fast[b]

    if not fast.all():
        g_all = np.asarray(inputs["gmatrix"], dtype=np.float32)
        W1 = np.asarray(inputs["W1"], dtype=np.float32)
        b1 = np.asarray(inputs["b1"], dtype=np.float32)
        W2 = np.asarray(inputs["W2"], dtype=np.float32)
        b2 = np.asarray(inputs["b2"], dtype=np.float32)
        for b in range(B):
            if not fast[b]:
                feat = np.ascontiguousarray(seg[b].T)
                r = _np_sample(feat, ep[b], g_all[b], W1, b1, W2, b2)
                out[b] = r + feat
    return out


# revision 4
# speedup vs baseline: 1.2862x; 1.1000x over previous
"""Trainium2 kernel for nn_BFM_torch_56384330662315 (gnn_message_passing).

Reference semantics (B=4, C=128, N=2048, K=16):
  feats = transpose(seg_features, (0,2,1))                 # [B,N,C]
  per sample: adj = boundary-cut symmetric kNN graph; two GCN layers
  out = refined + feats

Each GCN layer computes ``out = (adj/deg) @ f + feat`` but returns plain
``feat`` whenever any node has zero degree (``has_zero`` in the reference).
Any node classified as a boundary node (argmax(edge_preds)==1) gets its row
AND column zeroed in the symmetric adjacency, so it has zero degree.  Hence
for every sample that has at least one edge node and at least one non-edge
node, both GCN layers are exact identities and the whole module reduces,
bit-for-bit in fp32, to:

  out = 2 * transpose(seg_features, (0,2,1))

The device kernel evaluates that scaled transpose, data-parallel over the
8 NeuronCores.  Each core handles a [128, 1024] half-sample in bfloat16
(rel err ~1e-3, well under the 2e-2 gate):

  - one DMA-transpose (XBAR) load brings x^T into SBUF already transposed,
    split [6,2] blocks so the second transfer pipelines behind the first
    HWDGE descriptor generation;
  - DVE applies the x2 (the only arithmetic left);
  - two HWDGE stores write the result via 2KB/partition descriptors into a
    packed [128, 8*128] DRAM layout (the host unpermutes, a pure reshape).

The per-sample condition is checked on host from edge_preds (tiny);
samples that don't satisfy it (probability ~2^-2047 for the randn inputs
this problem is generated with) fall back to an exact numpy port of the
reference.
"""

import os

import numpy as np

# recover cleanly if a previous kernel left a NeuronCore exec unit wedged
os.environ.setdefault("NEURON_RT_RESET_CORES", "1")

B, C, N, K = 4, 128, 2048, 16
GEO_FILL = 1000.0
NCORES = 8
NSH = N * B // NCORES  # 1024 columns of seg_features per core
NBLK = NSH // 128      # 8 transpose blocks per core

# per-core program shape (sweep-tuned in TimelineSim)
SPLIT = [4, 4]

_EXEC = None  # cached (jitted shard_map, metadata)


def _bf16():
    import ml_dtypes

    return np.dtype(ml_dtypes.bfloat16)


def _build_nc():
    """Per-core program: y[128, 8*128] = packed 2*x[128,1024]^T in bf16.

    y[p, i*128 + c] = 2 * x[c, i*128 + p]  (host unpermutes blocks).

    Raw bass (no TileContext — its epilogue barrier costs ~500ns on the
    tail); manual semaphores order the three stages.  Pipeline
    (TimelineSim-tuned, [4,4] block splits): two XBAR dma-transpose loads
    (HWDGE descriptor generations serialize at ~625ns, so the second
    transfer starts right as the first ends), two DVE x2 scales (DVE is 3x
    faster than ACT here), two SP HWDGE stores whose descriptor
    generations pipeline exactly behind the scales.
    """
    from concourse import bacc, mybir

    BF16 = mybir.dt.bfloat16
    nc = bacc.Bacc(
        "TRN2",
        target_bir_lowering=False,
        debug=False,
        num_devices=NCORES,
    )
    x = nc.dram_tensor("x", [C, NSH], BF16, kind="ExternalInput").ap()
    y = nc.dram_tensor("y", [128, NSH], BF16, kind="ExternalOutput").ap()

    xt = nc.alloc_sbuf_tensor("xt_raw", [128, NBLK, 128], BF16).ap()
    ot = nc.alloc_sbuf_tensor("ot_raw", [128, NBLK, 128], BF16).ap()
    x_sem = nc.alloc_semaphore("x_sem")
    s_sem = nc.alloc_semaphore("s_sem")
    y_sem = nc.alloc_semaphore("y_sem")

    b0 = 0
    for nb in SPLIT:
        nc.sync.dma_start(
            xt[:, b0 : b0 + nb, :],
            x[:, b0 * 128 : (b0 + nb) * 128],
            transpose=True,
        ).then_inc(x_sem, 16)
        b0 += nb

    b0 = 0
    for j, nb in enumerate(SPLIT):
        nc.vector.wait_ge(x_sem, 16 * (j + 1))
        nc.vector.tensor_scalar_mul(
            ot[:, b0 : b0 + nb, :], xt[:, b0 : b0 + nb, :], 2.0
        ).then_inc(s_sem, 1)
        b0 += nb

    b0 = 0
    for j, nb in enumerate(SPLIT):
        nc.sync.wait_ge(s_sem, j + 1)
        nc.sync.dma_start(
            y[:, b0 * 128 : (b0 + nb) * 128],
            ot[:, b0 : b0 + nb, :],
        ).then_inc(y_sem, 16)
        b0 += nb

    nc.sync.wait_ge(y_sem, 16 * len(SPLIT))
    nc.compile()
    return nc


def _get_exec():
    """Build the per-core Bass program once and wrap it in a cached
    jit(shard_map) over the 8 cores (mirrors bass2jax.run_bass_via_pjrt)."""
    global _EXEC
    if _EXEC is not None:
        return _EXEC

    import jax
    from jax.experimental.shard_map import shard_map
    from jax.sharding import Mesh, PartitionSpec

    from concourse import bass2jax, mybir

    bass2jax.install_neuronx_cc_hook()
    nc = _build_nc()
    partition_name = nc.partition_id_tensor.name if nc.partition_id_tensor else None

    in_names: list = []
    out_names: list = []
    out_avals: list = []
    zero_outs: list = []
    for alloc in nc.m.functions[0].allocations:
        if not isinstance(alloc, mybir.MemoryLocationSet):
            continue
        name = alloc.memorylocations[0].name
        if alloc.kind == "ExternalInput":
            if name != partition_name:
                in_names.append(name)
        elif alloc.kind == "ExternalOutput":
            out_names.append(name)
            shape = tuple(alloc.tensor_shape)
            dtype = mybir.dt.np(alloc.dtype)
            out_avals.append(jax.core.ShapedArray(shape, dtype))
            zero_outs.append(np.zeros(shape, dtype))
    n_params = len(in_names)
    n_outs = len(out_avals)
    all_names = in_names + out_names
    if partition_name is not None:
        all_names.append(partition_name)

    def _body(*args):
        operands = list(args)
        if partition_name is not None:
            operands.append(bass2jax.partition_id_tensor())
        outs = bass2jax._bass_exec_p.bind(
            *operands,
            out_avals=tuple(out_avals),
            in_names=tuple(all_names),
            out_names=tuple(out_names),
            lowering_input_output_aliases=(),
            sim_require_finite=True,
            sim_require_nnan=True,
            nc=nc,
        )
        return tuple(outs)

    devices = jax.devices()[:NCORES]
    assert len(devices) == NCORES, f"need {NCORES} cores, have {len(jax.devices())}"
    mesh = Mesh(np.asarray(devices), ("core",))
    in_specs = (PartitionSpec("core"),) * (n_params + n_outs)
    out_specs = (PartitionSpec("core"),) * n_outs
    donate = tuple(range(n_params, n_params + n_outs))
    sharded = jax.jit(
        shard_map(
            _body, mesh=mesh, in_specs=in_specs, out_specs=out_specs, check_rep=False
        ),
        donate_argnums=donate,
        keep_unused=True,
    )
    _EXEC = (sharded, in_names, out_names, out_avals, zero_outs)
    return _EXEC


def _make_concat_inputs(seg: np.ndarray):
    """Per-core bf16 input shards, concatenated on axis 0 for shard_map."""
    bf16 = _bf16()
    xs = []
    for k in range(NCORES):
        b, h = k // 2, k % 2
        xs.append(seg[b, :, h * NSH : (h + 1) * NSH].astype(bf16))
    return {"x": np.concatenate(xs, axis=0)}


def _run_device(seg: np.ndarray) -> np.ndarray:
    """seg [B,C,N] f32 -> 2*transpose [B,N,C] f32 on the 8 cores (bf16
    internally, rel err ~1e-3), with retry and a host fallback in case a
    previous session left the accelerator wedged."""
    last_err = None
    for attempt in range(2):
        try:
            return _run_device_once(seg)
        except Exception as e:  # transient NRT_EXEC_UNIT_UNRECOVERABLE etc.
            last_err = e
    import sys

    print(
        f"kernel: device path failed twice ({type(last_err).__name__}: "
        f"{last_err}); computing on host",
        file=sys.stderr,
    )
    return np.ascontiguousarray(2.0 * seg.transpose(0, 2, 1))


def _run_device_once(seg: np.ndarray) -> np.ndarray:
    sharded, in_names, out_names, out_avals, zero_outs = _get_exec()
    by_name = _make_concat_inputs(seg)
    concat_in = [by_name[n] for n in in_names]
    concat_zeros = [
        np.zeros((NCORES * z.shape[0], *z.shape[1:]), z.dtype) for z in zero_outs
    ]
    out_arrs = sharded(*concat_in, *concat_zeros)
    y = np.asarray(out_arrs[out_names.index("y")]).reshape(NCORES, 128, NBLK, 128)

    out = np.empty((B, N, C), dtype=np.float32)
    for k in range(NCORES):
        b, h = k // 2, k % 2
        # y[k][p, i, c] = 2*x[c, i*128+p] -> rows i*128+p of the half-sample
        out[b, h * NSH : (h + 1) * NSH, :] = (
            y[k].transpose(1, 0, 2).reshape(NSH, C).astype(np.float32)
        )
    return out


# ---------------------------------------------------------------------------
# Exact numpy port of the reference — fallback for samples where the GCN does
# not collapse to identity (never hit for this problem's input distribution).
# ---------------------------------------------------------------------------


def _np_build_adj(g, edge_cls, k):
    n = g.shape[0]
    nbrs = np.argsort(g, axis=-1, kind="stable")[:, :k]
    rows = np.arange(n)[:, None]
    adj = np.zeros((n, n), g.dtype)
    adj[rows, nbrs] = 1.0
    adj[nbrs, rows] = 1.0
    is_edge = edge_cls == 1
    adj = np.where(is_edge[:, None], 0.0, adj)
    edge_col = is_edge[None, :]
    cond = (adj == 1) & edge_col
    maxgeo = np.min(np.where(cond, g, GEO_FILL), axis=-1)
    adjr = np.where(g > maxgeo[:, None], 0.0, adj)
    adjr = np.where(edge_col, 0.0, adjr)
    adj2 = np.where(is_edge[:, None], 0.0, adjr)
    adj_sym = ((adj2 > 0) | (adj2.T > 0)).astype(g.dtype)
    if np.all(is_edge):
        return np.eye(n, dtype=g.dtype)
    return adj_sym


def _np_gcn(feat, adj, W, b):
    identity = feat
    f = np.maximum(feat @ W.T + b, 0.0).astype(np.float32)
    row_deg = np.sum(adj, axis=-1, keepdims=True)
    col_deg = np.sum(adj, axis=-2, keepdims=True)
    degree = np.sqrt(row_deg) @ np.sqrt(col_deg)
    if np.any(degree == 0):
        return identity
    out = (adj / degree) @ f + identity
    return out.astype(np.float32)


def _np_sample(feat, ep, g, W1, b1, W2, b2):
    edge_cls = np.argmax(ep, axis=0)
    adj = _np_build_adj(g, edge_cls, K)
    r = _np_gcn(feat, adj, W1, b1)
    r = _np_gcn(r, adj, W2, b2)
    return r


def kernel(**inputs) -> np.ndarray:
    seg = np.ascontiguousarray(np.asarray(inputs["seg_features"], dtype=np.float32))
    ep = np.asarray(inputs["edge_preds"], dtype=np.float32)

    # argmax over the 2 class logits: class 1 iff ep[1] > ep[0] (ties -> 0)
    edge = ep[:, 1, :] > ep[:, 0, :]
    any_e = edge.any(axis=1)
    all_e = edge.all(axis=1)
    fast = any_e & ~all_e  # GCN layers are exact identities

    out = _run_device(seg)  # 2 * transpose, correct wherever fast[b]

    if not fast.all():
        g_all = np.asarray(inputs["gmatrix"], dtype=np.float32)
        W1 = np.asarray(inputs["W1"], dtype=np.float32)
        b1 = np.asarray(inputs["b1"], dtype=np.float32)
        W2 = np.asarray(inputs["W2"], dtype=np.float32)
        b2 = np.asarray(inputs["b2"], dtype=np.float32)
        for b in range(B):
            if not fast[b]:
                feat = np.ascontiguousarray(seg[b].T)
                r = _np_sample(feat, ep[b], g_all[b], W1, b1, W2, b2)
                out[b] = r + feat
    return out
